# revision 3
# baseline (speedup 1.0000x reference)
"""Trainium2 Bass kernel for windowed (local) causal self-attention.

Reference computation (per batch element, fp32):
    q = x @ Wq.T + bq ; k = x @ Wk.T + bk ; v = x @ Wv.T + bv
    per non-overlapping window of 256 tokens:
        attn = softmax(causal_mask(q k^T * HEAD_DIM**-0.5))
        out  = attn @ v
    o = out @ Wo.T + bo + x

Sharding: data-parallel over (batch, window): 64 window-blocks of 256
tokens -> 8 cores x 8 windows.  Weights replicated.

Per-core kernel strategy:
  - all four transposed weights (W.T, [e_in, e_out]) resident in SBUF as
    float32r (PE matmul dtype: 1 cycle/row at N>=256, fp32-equivalent
    numerics on TRN2 -- measured bit-identical to fp32 matmul).
  - per window: load x (256 tokens), PE-transpose to xT [E, 256];
    qT/kT = W.T^T @ xT in [e_out, t] layout (bias fused in ACT evac);
    scores = qT^T qK blocks with fused scale+exp+row-sum on ACT;
    normalized attn transposed back via PE; v token-major; out^T = v^T @
    attnT; o = outT^T @ Wo.T + bo (bias via K=1 ones-matmul) + x.
"""
import sys

sys.path.insert(0, "/opt/trn_rl_repo")

import numpy as np

import concourse.bass as bass
import concourse.bacc as bacc
import concourse.mybir as mybir
import concourse.tile as tile
from concourse.bass_utils import run_bass_kernel_spmd

F32 = mybir.dt.float32
F32R = mybir.dt.float32r
AF = mybir.ActivationFunctionType

E = 1024          # embed dim
ET = E // 128     # e-tiles
W = 256           # window size
NW = 8            # windows per core
T = NW * W        # tokens per core
N_CORES = 8
SCALE = (E // 16) ** (-0.5)  # HEAD_DIM ** -0.5 = 0.125
NEG = -1.0e30


def build_nc(nw=NW):
    t_core = nw * W
    nc = bacc.Bacc("TRN2", target_bir_lowering=False, debug=False)

    x_d = nc.dram_tensor("x", [t_core, E], F32R, kind="ExternalInput")
    w_d = {
        m: nc.dram_tensor(f"w{m}", [E, E], F32R, kind="ExternalInput")
        for m in ("q", "k", "v", "o")
    }
    bq_d = nc.dram_tensor("bq", [128, ET], F32, kind="ExternalInput")
    bk_d = nc.dram_tensor("bk", [128, ET], F32, kind="ExternalInput")
    bv_d = nc.dram_tensor("bv", [128, ET], F32, kind="ExternalInput")
    bo_d = nc.dram_tensor("bo", [1, E], F32R, kind="ExternalInput")
    o_d = nc.dram_tensor("o", [t_core, E], F32, kind="ExternalOutput")

    # host-side constants baked into the NEFF
    mask_np = np.zeros((2, 128, W), dtype=np.float32)
    for qt in range(2):
        r = np.arange(128)[:, None] + qt * 128
        c = np.arange(W)[None, :]
        mask_np[qt][c > r] = NEG
    mask_d = nc.inline_tensor(mask_np, "mask")
    ident_d = nc.inline_tensor(np.eye(128, dtype=np.float32), "ident")
    ones_d = nc.inline_tensor(np.ones((1, 128), dtype=np.float32), "ones")

    with tile.TileContext(nc) as tc:
        with (
            tc.tile_pool(name="wp", bufs=1) as wp,
            tc.tile_pool(name="cp", bufs=1) as cp,
            tc.tile_pool(name="xp", bufs=3) as xp,
            tc.tile_pool(name="xtp", bufs=1) as xtp,
            tc.tile_pool(name="qtp", bufs=1) as qtp,
            tc.tile_pool(name="ktp", bufs=1) as ktp,
            tc.tile_pool(name="otp", bufs=1) as otp,
            tc.tile_pool(name="vp", bufs=2) as vp,
            tc.tile_pool(name="sp", bufs=2) as sp,
            tc.tile_pool(name="ap_", bufs=2) as apool,
            tc.tile_pool(name="atp", bufs=2) as atp,
            tc.tile_pool(name="smp", bufs=8) as smp,
            tc.tile_pool(name="op", bufs=2) as op,
            tc.tile_pool(name="ps_qk", bufs=3, space=bass.MemorySpace.PSUM) as ps_qk,
            tc.tile_pool(name="ps_big", bufs=3, space=bass.MemorySpace.PSUM) as ps_big,
            tc.tile_pool(name="ps_tr", bufs=2, space=bass.MemorySpace.PSUM) as ps_tr,
        ):
            # ---- resident constants ----
            ident = cp.tile([128, 128], F32R, tag="ident")
            nc.sync.dma_start(ident[:], ident_d.ap().bitcast(F32R))
            masks = cp.tile([128, 2, W], F32, tag="mask")
            for qt in range(2):
                nc.sync.dma_start(masks[:, qt, :], mask_d.ap()[qt])
            ones = cp.tile([1, 128], F32R, tag="ones")
            nc.sync.dma_start(ones[:], ones_d.ap().bitcast(F32R))
            bo_sb = cp.tile([1, E], F32R, tag="bo")
            nc.sync.dma_start(bo_sb[:], bo_d.ap())
            bq_sb = cp.tile([128, ET], F32, tag="bq")
            nc.sync.dma_start(bq_sb[:], bq_d.ap())
            bk_sb = cp.tile([128, ET], F32, tag="bk")
            nc.sync.dma_start(bk_sb[:], bk_d.ap())
            bv_sb = cp.tile([128, ET], F32, tag="bv")
            nc.sync.dma_start(bv_sb[:], bv_d.ap())

            # ---- resident weights: wsb[m][p, ei, eo] = W_m.T[ei*128+p, eo] ----
            wsb = {}
            for m in ("q", "k", "v", "o"):
                wsb[m] = wp.tile([128, ET, E], F32R, tag=f"w{m}", name=f"w{m}sb")
                wr = w_d[m].ap().rearrange("(a p) n -> a p n", p=128)
                for ei in range(ET):
                    nc.sync.dma_start(wsb[m][:, ei, :], wr[ei])

            for w in range(nw):
                tok0 = w * W

                # ---- load x window (token-major), 2 tiles of [128, E] ----
                x_w = []
                for tt in range(2):
                    xt_ = xp.tile([128, E], F32R, tag="x")
                    nc.sync.dma_start(
                        xt_[:], x_d.ap()[tok0 + tt * 128 : tok0 + (tt + 1) * 128, :]
                    )
                    x_w.append(xt_)

                # ---- transpose -> xT[p, ei, t] (e-major) ----
                xT = xtp.tile([128, ET, W], F32R, tag="xT")
                for tt in range(2):
                    for eh in range(2):
                        ptr = ps_tr.tile([128, 512], F32R, tag="tr")
                        for j in range(4):
                            ei = eh * 4 + j
                            nc.tensor.transpose(
                                ptr[:, j * 128 : (j + 1) * 128],
                                x_w[tt][:, ei * 128 : (ei + 1) * 128],
                                ident[:],
                            )
                        nc.vector.tensor_copy(
                            xT[:, eh * 4 : eh * 4 + 4, tt * 128 : (tt + 1) * 128],
                            ptr[:].rearrange("p (a b) -> p a b", a=4),
                        )

                # ---- q/k projections -> [e_out, t] layout, bias fused ----
                qT = qtp.tile([128, ET, W], F32R, tag="qT")
                kT = ktp.tile([128, ET, W], F32R, tag="kT")
                for dst, m, b_sb in ((qT, "q", bq_sb), (kT, "k", bk_sb)):
                    for eo in range(ET):
                        pp = ps_qk.tile([128, W], F32, tag="qk")
                        for ei in range(ET):
                            nc.tensor.matmul(
                                pp[:],
                                wsb[m][:, ei, eo * 128 : (eo + 1) * 128],
                                xT[:, ei, :],
                                start=(ei == 0),
                                stop=(ei == ET - 1),
                            )
                        nc.scalar.add(dst[:, eo, :], pp[:], b_sb[:, eo : eo + 1])

                # ---- scores + softmax + transpose(attn) ----
                aT = []
                for ktt in range(2):
                    t_ = atp.tile([128, W], F32R, tag="aT", name=f"aT{ktt}")
                    aT.append(t_)
                for qt in range(2):
                    sc = ps_qk.tile([128, W], F32, tag="qk")
                    for ei in range(ET):
                        nc.tensor.matmul(
                            sc[:],
                            qT[:, ei, qt * 128 : (qt + 1) * 128],
                            kT[:, ei, :],
                            start=(ei == 0),
                            stop=(ei == ET - 1),
                        )
                    s_sb = sp.tile([128, W], F32, tag="s")
                    nc.vector.tensor_add(s_sb[:], sc[:], masks[:, qt, :])
                    e_sb = sp.tile([128, W], F32, tag="e")
                    sums = smp.tile([128, 1], F32, tag="sum")
                    nc.scalar.activation(
                        e_sb[:], s_sb[:], AF.Exp, scale=SCALE, accum_out=sums[:]
                    )
                    rec = smp.tile([128, 1], F32, tag="rec")
                    nc.vector.reciprocal(rec[:], sums[:])
                    a_sb = apool.tile([128, W], F32R, tag="a")
                    nc.vector.tensor_scalar_mul(a_sb[:], e_sb[:], rec[:])
                    # transpose attn block rows->cols: aT[ktt][:, qt*128:...]
                    for ktt in range(2):
                        ptr = ps_tr.tile([128, 128], F32R, tag="tr", name="ptra")
                        nc.tensor.transpose(
                            ptr[:], a_sb[:, ktt * 128 : (ktt + 1) * 128], ident[:]
                        )
                        nc.vector.tensor_copy(
                            aT[ktt][:, qt * 128 : (qt + 1) * 128], ptr[:]
                        )

                # ---- v projection (token-major) ----
                v_w = []
                for tt in range(2):
                    vt = vp.tile([128, E], F32R, tag="v")
                    for eoh in range(2):
                        pv = ps_big.tile([128, 512], F32, tag="big")
                        for ei in range(ET):
                            nc.tensor.matmul(
                                pv[:],
                                xT[:, ei, tt * 128 : (tt + 1) * 128],
                                wsb["v"][:, ei, eoh * 512 : (eoh + 1) * 512],
                                start=(ei == 0),
                                stop=(ei == ET - 1),
                            )
                        nc.vector.tensor_copy(vt[:, eoh * 512 : (eoh + 1) * 512], pv[:])
                    v_w.append(vt)

                # ---- attn @ v -> outT [e, t] layout, bias bv fused ----
                outT = otp.tile([128, ET, W], F32R, tag="outT")
                for et in range(ET):
                    pa = ps_qk.tile([128, W], F32, tag="qk")
                    for ktt in range(2):
                        nc.tensor.matmul(
                            pa[:],
                            v_w[ktt][:, et * 128 : (et + 1) * 128],
                            aT[ktt][:],
                            start=(ktt == 0),
                            stop=(ktt == 1),
                        )
                    nc.scalar.add(outT[:, et, :], pa[:], bv_sb[:, et : et + 1])

                # ---- output projection + bo + residual ----
                for tt in range(2):
                    for eoh in range(2):
                        po = ps_big.tile([128, 512], F32, tag="big")
                        for ei in range(ET):
                            nc.tensor.matmul(
                                po[:],
                                outT[:, ei, tt * 128 : (tt + 1) * 128],
                                wsb["o"][:, ei, eoh * 512 : (eoh + 1) * 512],
                                start=(ei == 0),
                                stop=False,
                            )
                        nc.tensor.matmul(
                            po[:],
                            ones[:],
                            bo_sb[:, eoh * 512 : (eoh + 1) * 512],
                            start=False,
                            stop=True,
                        )
                        o_sb = op.tile([128, 512], F32, tag="o")
                        nc.vector.tensor_add(
                            o_sb[:],
                            po[:],
                            x_w[tt][:, eoh * 512 : (eoh + 1) * 512].bitcast(F32),
                        )
                        nc.sync.dma_start(
                            o_d.ap()[
                                tok0 + tt * 128 : tok0 + (tt + 1) * 128,
                                eoh * 512 : (eoh + 1) * 512,
                            ],
                            o_sb[:],
                        )

    nc.compile()
    return nc


_NC_CACHE = {}


def _get_nc(nw=NW):
    if nw not in _NC_CACHE:
        _NC_CACHE[nw] = build_nc(nw)
    return _NC_CACHE[nw]


def kernel(x, Wq, bq, Wk, bk, Wv, bv, Wo, bo):
    x = np.asarray(x, dtype=np.float32)
    B, S, _ = x.shape
    x_flat = np.ascontiguousarray(x.reshape(B * S, E))
    t_core = B * S // N_CORES
    assert t_core == T

    common = {
        "wq": np.ascontiguousarray(np.asarray(Wq, np.float32).T),
        "wk": np.ascontiguousarray(np.asarray(Wk, np.float32).T),
        "wv": np.ascontiguousarray(np.asarray(Wv, np.float32).T),
        "wo": np.ascontiguousarray(np.asarray(Wo, np.float32).T),
        "bq": np.ascontiguousarray(np.asarray(bq, np.float32).reshape(ET, 128).T),
        "bk": np.ascontiguousarray(np.asarray(bk, np.float32).reshape(ET, 128).T),
        "bv": np.ascontiguousarray(np.asarray(bv, np.float32).reshape(ET, 128).T),
        "bo": np.ascontiguousarray(np.asarray(bo, np.float32).reshape(1, E)),
    }
    in_maps = [
        {"x": np.ascontiguousarray(x_flat[i * t_core : (i + 1) * t_core]), **common}
        for i in range(N_CORES)
    ]

    nc = _get_nc()
    res = run_bass_kernel_spmd(nc, in_maps, core_ids=list(range(N_CORES)))
    out = np.concatenate([res.results[i]["o"] for i in range(N_CORES)], axis=0)
    return out.reshape(B, S, E).astype(np.float32)


# revision 4
# speedup vs baseline: 1.1225x; 1.1225x over previous
"""Trainium2 Bass kernel for windowed (local) causal self-attention.

Reference computation (per batch element, fp32):
    q = x @ Wq.T + bq ; k = x @ Wk.T + bk ; v = x @ Wv.T + bv
    per non-overlapping window of 256 tokens:
        attn = softmax(causal_mask(q k^T * HEAD_DIM**-0.5))
        out  = attn @ v
    o = out @ Wo.T + bo + x

Sharding: data-parallel over (batch, window): 64 window-blocks of 256
tokens -> 8 cores x 8 windows.  Weights replicated.

Per-core kernel strategy:
  - all four transposed weights (W.T, [e_in, e_out]) resident in SBUF as
    float32r (PE matmul dtype: 1 cycle/row at N>=256, fp32-equivalent
    numerics on TRN2 -- measured bit-identical to fp32 matmul).
  - per window: load x (256 tokens), PE-transpose to xT [E, 256];
    qT/kT = W.T^T @ xT in [e_out, t] layout (bias fused in ACT evac);
    scores = qT^T qK blocks with fused scale+exp+row-sum on ACT;
    normalized attn transposed back via PE; v token-major; out^T = v^T @
    attnT; o = outT^T @ Wo.T + bo (bias via K=1 ones-matmul) + x.
"""
import sys

sys.path.insert(0, "/opt/trn_rl_repo")

import numpy as np

import concourse.bass as bass
import concourse.bacc as bacc
import concourse.mybir as mybir
import concourse.tile as tile
from concourse.bass_utils import run_bass_kernel_spmd

F32 = mybir.dt.float32
F32R = mybir.dt.float32r
AF = mybir.ActivationFunctionType

E = 1024          # embed dim
ET = E // 128     # e-tiles
W = 256           # window size
NW = 8            # windows per core
T = NW * W        # tokens per core
N_CORES = 8
SCALE = (E // 16) ** (-0.5)  # HEAD_DIM ** -0.5 = 0.125
NEG = -1.0e30


def build_nc(nw=NW):
    t_core = nw * W
    nc = bacc.Bacc("TRN2", target_bir_lowering=False, debug=False)

    x_d = nc.dram_tensor("x", [t_core, E], F32R, kind="ExternalInput")
    w_d = {
        m: nc.dram_tensor(f"w{m}", [E, E], F32R, kind="ExternalInput")
        for m in ("q", "k", "v", "o")
    }
    bq_d = nc.dram_tensor("bq", [128, ET], F32, kind="ExternalInput")
    bk_d = nc.dram_tensor("bk", [128, ET], F32, kind="ExternalInput")
    bv_d = nc.dram_tensor("bv", [128, ET], F32, kind="ExternalInput")
    bo_d = nc.dram_tensor("bo", [1, E], F32R, kind="ExternalInput")
    o_d = nc.dram_tensor("o", [t_core, E], F32, kind="ExternalOutput")

    # host-side constants baked into the NEFF
    mask_np = np.zeros((2, 128, W), dtype=np.float32)
    for qt in range(2):
        r = np.arange(128)[:, None] + qt * 128
        c = np.arange(W)[None, :]
        mask_np[qt][c > r] = NEG
    mask_d = nc.inline_tensor(mask_np, "mask")
    ident_d = nc.inline_tensor(np.eye(128, dtype=np.float32), "ident")
    ones_d = nc.inline_tensor(np.ones((1, 128), dtype=np.float32), "ones")

    with tile.TileContext(nc) as tc:
        with (
            tc.tile_pool(name="wp", bufs=1) as wp,
            tc.tile_pool(name="cp", bufs=1) as cp,
            tc.tile_pool(name="xp", bufs=3) as xp,
            tc.tile_pool(name="xtp", bufs=1) as xtp,
            tc.tile_pool(name="qtp", bufs=1) as qtp,
            tc.tile_pool(name="ktp", bufs=1) as ktp,
            tc.tile_pool(name="otp", bufs=1) as otp,
            tc.tile_pool(name="vp", bufs=2) as vp,
            tc.tile_pool(name="sp", bufs=2) as sp,
            tc.tile_pool(name="ap_", bufs=2) as apool,
            tc.tile_pool(name="atp", bufs=2) as atp,
            tc.tile_pool(name="smp", bufs=8) as smp,
            tc.tile_pool(name="op", bufs=2) as op,
            tc.tile_pool(name="ps_qk", bufs=3, space=bass.MemorySpace.PSUM) as ps_qk,
            tc.tile_pool(name="ps_big", bufs=3, space=bass.MemorySpace.PSUM) as ps_big,
            tc.tile_pool(name="ps_tr", bufs=2, space=bass.MemorySpace.PSUM) as ps_tr,
        ):
            # ---- resident constants ----
            ident = cp.tile([128, 128], F32R, tag="ident")
            nc.sync.dma_start(ident[:], ident_d.ap().bitcast(F32R))
            masks = cp.tile([128, 2, W], F32, tag="mask")
            for qt in range(2):
                nc.sync.dma_start(masks[:, qt, :], mask_d.ap()[qt])
            ones = cp.tile([1, 128], F32R, tag="ones")
            nc.sync.dma_start(ones[:], ones_d.ap().bitcast(F32R))
            bo_sb = cp.tile([1, E], F32R, tag="bo")
            nc.sync.dma_start(bo_sb[:], bo_d.ap())
            bq_sb = cp.tile([128, ET], F32, tag="bq")
            nc.sync.dma_start(bq_sb[:], bq_d.ap())
            bk_sb = cp.tile([128, ET], F32, tag="bk")
            nc.sync.dma_start(bk_sb[:], bk_d.ap())
            bv_sb = cp.tile([128, ET], F32, tag="bv")
            nc.sync.dma_start(bv_sb[:], bv_d.ap())

            # ---- resident weights: wsb[m][p, ei, eo] = W_m.T[ei*128+p, eo] ----
            # Weight DMAs are interleaved into window 0's emission below so
            # the sync engine starts x/window work immediately instead of
            # serializing 16MB of weight loads ahead of all compute.
            wsb = {}
            for m in ("q", "k", "v", "o"):
                wsb[m] = wp.tile([128, ET, E], F32R, tag=f"w{m}", name=f"w{m}sb")

            def load_weight(m):
                wr = w_d[m].ap().rearrange("(a p) n -> a p n", p=128)
                for ei in range(ET):
                    nc.sync.dma_start(wsb[m][:, ei, :], wr[ei])

            for w in range(nw):
                tok0 = w * W

                # ---- load x window (token-major), 2 tiles of [128, E] ----
                x_w = []
                for tt in range(2):
                    xt_ = xp.tile([128, E], F32R, tag="x")
                    nc.sync.dma_start(
                        xt_[:], x_d.ap()[tok0 + tt * 128 : tok0 + (tt + 1) * 128, :]
                    )
                    x_w.append(xt_)
                if w == 0:
                    load_weight("q")

                # ---- transpose -> xT[p, ei, t] (e-major) ----
                xT = xtp.tile([128, ET, W], F32R, tag="xT")
                for tt in range(2):
                    for eh in range(2):
                        ptr = ps_tr.tile([128, 512], F32R, tag="tr")
                        for j in range(4):
                            ei = eh * 4 + j
                            nc.tensor.transpose(
                                ptr[:, j * 128 : (j + 1) * 128],
                                x_w[tt][:, ei * 128 : (ei + 1) * 128],
                                ident[:],
                            )
                        nc.vector.tensor_copy(
                            xT[:, eh * 4 : eh * 4 + 4, tt * 128 : (tt + 1) * 128],
                            ptr[:].rearrange("p (a b) -> p a b", a=4),
                        )

                if w == 0:
                    load_weight("k")

                # ---- q/k projections -> [e_out, t] layout, bias fused ----
                qT = qtp.tile([128, ET, W], F32R, tag="qT")
                kT = ktp.tile([128, ET, W], F32R, tag="kT")
                for dst, m, b_sb in ((qT, "q", bq_sb), (kT, "k", bk_sb)):
                    for eo in range(ET):
                        pp = ps_qk.tile([128, W], F32, tag="qk")
                        for ei in range(ET):
                            nc.tensor.matmul(
                                pp[:],
                                wsb[m][:, ei, eo * 128 : (eo + 1) * 128],
                                xT[:, ei, :],
                                start=(ei == 0),
                                stop=(ei == ET - 1),
                            )
                        nc.scalar.add(dst[:, eo, :], pp[:], b_sb[:, eo : eo + 1])

                if w == 0:
                    load_weight("v")

                # ---- scores + softmax + transpose(attn) ----
                aT = []
                for ktt in range(2):
                    t_ = atp.tile([128, W], F32R, tag="aT", name=f"aT{ktt}")
                    aT.append(t_)
                for qt in range(2):
                    sc = ps_qk.tile([128, W], F32, tag="qk")
                    for ei in range(ET):
                        nc.tensor.matmul(
                            sc[:],
                            qT[:, ei, qt * 128 : (qt + 1) * 128],
                            kT[:, ei, :],
                            start=(ei == 0),
                            stop=(ei == ET - 1),
                        )
                    s_sb = sp.tile([128, W], F32, tag="s")
                    nc.vector.tensor_add(s_sb[:], sc[:], masks[:, qt, :])
                    e_sb = sp.tile([128, W], F32, tag="e")
                    sums = smp.tile([128, 1], F32, tag="sum")
                    nc.scalar.activation(
                        e_sb[:], s_sb[:], AF.Exp, scale=SCALE, accum_out=sums[:]
                    )
                    rec = smp.tile([128, 1], F32, tag="rec")
                    nc.vector.reciprocal(rec[:], sums[:])
                    a_sb = apool.tile([128, W], F32R, tag="a")
                    nc.vector.tensor_scalar_mul(a_sb[:], e_sb[:], rec[:])
                    # transpose attn block rows->cols: aT[ktt][:, qt*128:...]
                    for ktt in range(2):
                        ptr = ps_tr.tile([128, 128], F32R, tag="tr", name="ptra")
                        nc.tensor.transpose(
                            ptr[:], a_sb[:, ktt * 128 : (ktt + 1) * 128], ident[:]
                        )
                        nc.vector.tensor_copy(
                            aT[ktt][:, qt * 128 : (qt + 1) * 128], ptr[:]
                        )

                if w == 0:
                    load_weight("o")

                # ---- v projection (token-major) ----
                v_w = []
                for tt in range(2):
                    vt = vp.tile([128, E], F32R, tag="v")
                    for eoh in range(2):
                        pv = ps_big.tile([128, 512], F32, tag="big")
                        for ei in range(ET):
                            nc.tensor.matmul(
                                pv[:],
                                xT[:, ei, tt * 128 : (tt + 1) * 128],
                                wsb["v"][:, ei, eoh * 512 : (eoh + 1) * 512],
                                start=(ei == 0),
                                stop=(ei == ET - 1),
                            )
                        nc.vector.tensor_copy(vt[:, eoh * 512 : (eoh + 1) * 512], pv[:])
                    v_w.append(vt)

                # ---- attn @ v -> outT [e, t] layout, bias bv fused ----
                outT = otp.tile([128, ET, W], F32R, tag="outT")
                for et in range(ET):
                    pa = ps_qk.tile([128, W], F32, tag="qk")
                    for ktt in range(2):
                        nc.tensor.matmul(
                            pa[:],
                            v_w[ktt][:, et * 128 : (et + 1) * 128],
                            aT[ktt][:],
                            start=(ktt == 0),
                            stop=(ktt == 1),
                        )
                    nc.scalar.add(outT[:, et, :], pa[:], bv_sb[:, et : et + 1])

                # ---- output projection + bo + residual ----
                for tt in range(2):
                    for eoh in range(2):
                        po = ps_big.tile([128, 512], F32, tag="big")
                        for ei in range(ET):
                            nc.tensor.matmul(
                                po[:],
                                outT[:, ei, tt * 128 : (tt + 1) * 128],
                                wsb["o"][:, ei, eoh * 512 : (eoh + 1) * 512],
                                start=(ei == 0),
                                stop=False,
                            )
                        nc.tensor.matmul(
                            po[:],
                            ones[:],
                            bo_sb[:, eoh * 512 : (eoh + 1) * 512],
                            start=False,
                            stop=True,
                        )
                        o_sb = op.tile([128, 512], F32, tag="o")
                        nc.vector.tensor_add(
                            o_sb[:],
                            po[:],
                            x_w[tt][:, eoh * 512 : (eoh + 1) * 512].bitcast(F32),
                        )
                        nc.sync.dma_start(
                            o_d.ap()[
                                tok0 + tt * 128 : tok0 + (tt + 1) * 128,
                                eoh * 512 : (eoh + 1) * 512,
                            ],
                            o_sb[:],
                        )

    nc.compile()
    return nc


_NC_CACHE = {}


def _get_nc(nw=NW):
    if nw not in _NC_CACHE:
        _NC_CACHE[nw] = build_nc(nw)
    return _NC_CACHE[nw]


def kernel(x, Wq, bq, Wk, bk, Wv, bv, Wo, bo):
    x = np.asarray(x, dtype=np.float32)
    B, S, _ = x.shape
    x_flat = np.ascontiguousarray(x.reshape(B * S, E))
    t_core = B * S // N_CORES
    assert t_core == T

    common = {
        "wq": np.ascontiguousarray(np.asarray(Wq, np.float32).T),
        "wk": np.ascontiguousarray(np.asarray(Wk, np.float32).T),
        "wv": np.ascontiguousarray(np.asarray(Wv, np.float32).T),
        "wo": np.ascontiguousarray(np.asarray(Wo, np.float32).T),
        "bq": np.ascontiguousarray(np.asarray(bq, np.float32).reshape(ET, 128).T),
        "bk": np.ascontiguousarray(np.asarray(bk, np.float32).reshape(ET, 128).T),
        "bv": np.ascontiguousarray(np.asarray(bv, np.float32).reshape(ET, 128).T),
        "bo": np.ascontiguousarray(np.asarray(bo, np.float32).reshape(1, E)),
    }
    in_maps = [
        {"x": np.ascontiguousarray(x_flat[i * t_core : (i + 1) * t_core]), **common}
        for i in range(N_CORES)
    ]

    nc = _get_nc()
    res = run_bass_kernel_spmd(nc, in_maps, core_ids=list(range(N_CORES)))
    out = np.concatenate([res.results[i]["o"] for i in range(N_CORES)], axis=0)
    return out.reshape(B, S, E).astype(np.float32)
